# revision 6
# baseline (speedup 1.0000x reference)
"""BitLinear forward (RMSNorm -> int8 activation quant -> ternary weight quant
-> matmul -> rescale) on 8 Trainium2 NeuronCores.

Sharding: data-parallel over rows. x (4,4096,1024) flattens to (16384,1024);
each core gets 2048 rows and the full weight. The weight is passed to the
device pre-transposed on the host (pure layout change): wT = weight.T with
shape (1024, 4096), so k lands on SBUF partitions and no on-device weight
transposes are needed. w_scale = mean|w| comes from a per-core 128-row shard
of wT followed by a scalar AllReduce over the 8 cores. Output is written as
bf16 (exact integers * f32 scale, bf16-rounded; ~0.3% of the error budget)
and upcast to f32 on the host.

Mixed-precision matmul: x_q are exact integers in [-128,127], w_t in
{-1,0,1}. k-chunks 0..3 (k<512) are cast to fp8e4 (one RNE rounding of the
integer x_q; w_t is exact in fp8) and run as DoubleRow perf-mode matmuls
(2 k-rows per PE pass -> 2x throughput); k-chunks 4..7 stay bf16 (exact).
Measured against the fixed-seed reference: rel_err ~1.76e-2 < 2e-2.

Ternarize: w_t = Sign(w) * (w^2 > (0.5*(w_scale+eps))^2), split so that only
two cheap DVE ops per chunk depend on the AllReduce; Sign/Square run as the
weight streams in. Round-half-to-even for x_q is fp32 magic-constant
(1.5*2^23) add/subtract. x transposes to [k, r] layout are identity matmuls
batched 4 chunks per PSUM bank with one wide copy back to SBUF.
"""

import numpy as np

import concourse.bass as bass
import concourse.mybir as mybir
import concourse.tile as tile
from concourse import bacc
from concourse.bass_utils import run_bass_kernel_spmd
from concourse.masks import make_identity
from concourse import bass_isa

F32 = mybir.dt.float32
BF16 = mybir.dt.bfloat16
FP8 = mybir.dt.float8e4
ALU = mybir.AluOpType
AF = mybir.ActivationFunctionType
DR = mybir.MatmulPerfMode.DoubleRow

N_CORES = 8
R_FULL, K, N = 16384, 1024, 4096
R = R_FULL // N_CORES          # 2048 rows per core
RT = R // 128                  # 16 row tiles per core
KC = K // 128                  # 8 k-chunks
KC8 = 4                        # chunks 0..3 are fp8 (DoubleRow), 4..7 bf16
NH = 2                         # n halves (2048 each)
NQ = 4                         # 512-wide psum tiles per half

C_MAGIC = 12582912.0           # 1.5 * 2^23: fp32 round-to-nearest-even trick
Q_EPS = 1e-5
NORM_EPS = 1e-6


def build_nc(g_is_ones: bool):
    nc = bacc.Bacc("TRN2", target_bir_lowering=False)

    x_d = nc.dram_tensor("x", [R, K], F32, kind="ExternalInput")
    wt_d = nc.dram_tensor("wt", [K, N], F32, kind="ExternalInput")
    wsh_d = nc.dram_tensor("wshard", [128, N], F32, kind="ExternalInput")
    cc_in = nc.dram_tensor("cc_in", [1, 1], F32)
    cc_out = nc.dram_tensor("cc_out", [1, 1], F32, addr_space="Shared")
    if not g_is_ones:
        g_d = nc.dram_tensor("g", [1, K], F32, kind="ExternalInput")
    out_d = nc.dram_tensor("out", [R, N], BF16, kind="ExternalOutput")

    with tile.TileContext(nc) as tc:
        with (
            tc.tile_pool(name="persist", bufs=1) as persist,
            tc.tile_pool(name="wkp", bufs=2) as wk_pool,
            tc.tile_pool(name="wscr", bufs=2) as wscr_pool,
            tc.tile_pool(name="xp", bufs=3) as x_pool,
            tc.tile_pool(name="xqTp", bufs=16) as xqT_pool,
            tc.tile_pool(name="csp", bufs=16) as cs_pool,
            tc.tile_pool(name="big", bufs=2) as big_pool,
            tc.tile_pool(name="stats", bufs=4) as st_pool,
            tc.tile_pool(name="osbp", bufs=3) as osb_pool,
            tc.tile_pool(name="pmm", bufs=6, space="PSUM") as psum_mm,
            tc.tile_pool(name="ptp", bufs=2, space="PSUM") as psum_tp,
        ):
            # ---- constants ----
            ident = persist.tile([128, 128], BF16, tag="ident")
            make_identity(nc, ident[:])
            cb = persist.tile([128, 1], F32, tag="cb")
            nc.vector.memset(cb[:], C_MAGIC)

            if not g_is_ones:
                g_row = persist.tile([1, K], F32, tag="g_row")
                nc.sync.dma_start(g_row[:], g_d[:])
                g_b = persist.tile([128, K], F32, tag="g_b")
                nc.gpsimd.partition_broadcast(g_b[:], g_row[0:1, :])

            # ternarized w^T, k on partitions: chunks 0..3 fp8, 4..7 bf16
            wT8 = persist.tile([128, KC8, N], FP8, tag="wT8", name="wT8")
            wTb = persist.tile([128, KC - KC8, N], BF16, tag="wTb", name="wTb")

            # ---- w_scale sum from the 128-row wT shard; AllReduce on the
            # gpsimd queue so its latency blocks nothing else ----
            with nc.named_scope("w_scale"):
                wsha = wk_pool.tile([128, N], F32, tag="wk", name="wsha")
                nc.gpsimd.dma_start(wsha[:], wsh_d[:])
                wabs = wscr_pool.tile([128, N], F32, tag="w2", name="wabs")
                wpart = st_pool.tile([128, 1], F32, tag="wpart")
                nc.scalar.activation(
                    wabs[:], wsha[:], AF.Abs, accum_out=wpart[:])
                wall = st_pool.tile([128, 1], F32, tag="wall")
                nc.gpsimd.partition_all_reduce(
                    wall[:], wpart[:], channels=128,
                    reduce_op=bass_isa.ReduceOp.add)
                nc.gpsimd.dma_start(cc_in[:], wall[0:1, :])
                nc.gpsimd.collective_compute(
                    "AllReduce", ALU.add,
                    replica_groups=[list(range(N_CORES))],
                    ins=[cc_in[:]], outs=[cc_out[:]],
                )
                wsum_g1 = st_pool.tile([1, 1], F32, tag="wsum_g1")
                nc.gpsimd.dma_start(wsum_g1[:], cc_out[:])
                wsum_g = st_pool.tile([128, 1], F32, tag="wsum_g")
                nc.gpsimd.partition_broadcast(wsum_g[:], wsum_g1[0:1, :])

            # ---- w chunk stage 1 (no AllReduce dependency):
            # DMA, w2 = w*w (DVE), sgn = Sign(w) (ACT, exact in fp8) ----
            w2_tiles = {}
            sgn_tiles = {}
            wk_tiles = {}

            def emit_w_pre(kt):
                with nc.named_scope("w_pre"):
                    wk = wk_pool.tile([128, N], F32, tag="wk", name=f"wk{kt}")
                    nc.sync.dma_start(wk[:], wt_d[kt * 128:(kt + 1) * 128, :])
                    w2 = wscr_pool.tile([128, N], F32, tag="w2", name=f"w2_{kt}")
                    nc.vector.tensor_mul(w2[:], wk[:], wk[:])
                    sgn = wscr_pool.tile([128, N], FP8, tag="sgn", name=f"sgn{kt}")
                    nc.scalar.activation(sgn[:], wk[:], AF.Sign)
                    w2_tiles[kt] = w2
                    sgn_tiles[kt] = sgn

            # ---- scale scalars (first DVE ops that wait on the AllReduce) ----
            def emit_scales():
                # w_scale = mean|w|; thr2 = (0.5*(w_scale+eps))^2
                wsb = persist.tile([128, 1], F32, tag="wsb")
                nc.vector.tensor_scalar(
                    out=wsb[:], in0=wsum_g[:], scalar1=1.0 / (N * K),
                    scalar2=None, op0=ALU.mult)
                speps = st_pool.tile([128, 1], F32, tag="speps")
                nc.vector.tensor_scalar(
                    out=speps[:], in0=wsum_g[:], scalar1=1.0 / (N * K),
                    scalar2=Q_EPS, op0=ALU.mult, op1=ALU.add)
                half = st_pool.tile([128, 1], F32, tag="half")
                nc.vector.tensor_scalar(
                    out=half[:], in0=speps[:], scalar1=0.5,
                    scalar2=None, op0=ALU.mult)
                thr2 = persist.tile([128, 1], F32, tag="thr2")
                nc.vector.tensor_mul(thr2[:], half[:], half[:])
                return wsb, thr2

            # ---- w chunk stage 2 (needs thr2): mask + combine on DVE ----
            def emit_w_post(kt, thr2):
                with nc.named_scope("w_tern"):
                    mask = wscr_pool.tile([128, N], FP8, tag="mask",
                                          name=f"mask{kt}")
                    nc.vector.tensor_scalar(
                        out=mask[:], in0=w2_tiles[kt][:],
                        scalar1=thr2[:, 0:1], scalar2=None, op0=ALU.is_gt)
                    if kt < KC8:
                        dst = wT8[:, kt, :]
                    else:
                        dst = wTb[:, kt - KC8, :]
                    nc.vector.tensor_mul(dst, mask[:], sgn_tiles[kt][:])

            # ---- x quantization (independent of w) ----
            xq8T_tiles = []
            xqbT_tiles = []
            xsc_tiles = []
            cs_tiles = []

            def emit_x(rt):
                with nc.named_scope("x_quant"):
                    xt = x_pool.tile([128, K], F32, tag="xt", name=f"xt{rt}")
                    nc.scalar.dma_start(xt[:], x_d[rt * 128:(rt + 1) * 128, :])

                    if g_is_ones:
                        xg = xt
                    else:
                        xg = big_pool.tile([128, K], F32, tag="xg", name=f"xg{rt}")
                        nc.vector.tensor_mul(xg[:], xt[:], g_b[:])

                    xsq = big_pool.tile([128, K], FP8, tag="xsq", name=f"xsq{rt}")
                    ssq = st_pool.tile([128, 1], F32, tag="ssq")
                    nc.vector.scalar_tensor_tensor(
                        out=xsq[:], in0=xt[:], scalar=1.0, in1=xt[:],
                        op0=ALU.mult, op1=ALU.mult, accum_out=ssq[:])
                    am = st_pool.tile([128, 1], F32, tag="am")
                    nc.vector.tensor_reduce(
                        am[:], xg[:], axis=mybir.AxisListType.X, op=ALU.max,
                        apply_absolute_value=True)

                    # rs = 1/sqrt(ms + eps) with one Newton step on sqrt
                    ms = st_pool.tile([128, 1], F32, tag="ms")
                    nc.vector.tensor_scalar(
                        out=ms[:], in0=ssq[:], scalar1=1.0 / K,
                        scalar2=NORM_EPS, op0=ALU.mult, op1=ALU.add)
                    s0 = st_pool.tile([128, 1], F32, tag="s0")
                    nc.scalar.sqrt(s0[:], ms[:])
                    r0 = st_pool.tile([128, 1], F32, tag="r0")
                    nc.vector.reciprocal(r0[:], s0[:])
                    t0 = st_pool.tile([128, 1], F32, tag="t0")
                    nc.vector.tensor_mul(t0[:], ms[:], r0[:])
                    t1 = st_pool.tile([128, 1], F32, tag="t1")
                    nc.vector.tensor_add(t1[:], t0[:], s0[:])
                    s1 = st_pool.tile([128, 1], F32, tag="s1")
                    nc.vector.tensor_scalar(
                        out=s1[:], in0=t1[:], scalar1=0.5,
                        scalar2=None, op0=ALU.mult)
                    rs = st_pool.tile([128, 1], F32, tag="rs")
                    nc.vector.reciprocal(rs[:], s1[:])

                    axr = st_pool.tile([128, 1], F32, tag="axr")
                    nc.vector.tensor_mul(axr[:], am[:], rs[:])
                    xsc = cs_pool.tile([128, 1], F32, tag="xsc", name=f"xsc{rt}")
                    nc.vector.tensor_scalar(
                        out=xsc[:], in0=axr[:], scalar1=1.0 / 127.0,
                        scalar2=None, op0=ALU.mult)
                    sx = st_pool.tile([128, 1], F32, tag="sx")
                    nc.vector.tensor_scalar(
                        out=sx[:], in0=axr[:], scalar1=1.0 / 127.0,
                        scalar2=Q_EPS, op0=ALU.mult, op1=ALU.add)
                    dx = st_pool.tile([128, 1], F32, tag="dx")
                    nc.vector.reciprocal(dx[:], sx[:])
                    srow = st_pool.tile([128, 1], F32, tag="srow")
                    nc.vector.tensor_mul(srow[:], rs[:], dx[:])

                    # x_q = RNE(xg * srow) via +C (ACT), then -C on DVE which
                    # also converts: chunks 0..3 -> fp8e4, chunks 4..7 -> bf16
                    ux = big_pool.tile([128, K], F32, tag="ux", name=f"ux{rt}")
                    nc.scalar.activation(
                        ux[:], xg[:], AF.Identity,
                        bias=cb[:, 0:1], scale=srow[:, 0:1])
                    xq8 = big_pool.tile([128, K // 2], FP8, tag="xq8",
                                        name=f"xq8{rt}")
                    nc.vector.tensor_scalar(
                        out=xq8[:], in0=ux[:, :K // 2], scalar1=C_MAGIC,
                        scalar2=None, op0=ALU.subtract)
                    xqb = big_pool.tile([128, K // 2], BF16, tag="xqb",
                                        name=f"xqb{rt}")
                    nc.vector.tensor_scalar(
                        out=xqb[:], in0=ux[:, K // 2:], scalar1=C_MAGIC,
                        scalar2=None, op0=ALU.subtract)

                    # transpose via identity matmuls, 4 chunks per psum bank
                    xq8T = xqT_pool.tile([128, KC8, 128], FP8, tag="xq8T",
                                         name=f"xq8T{rt}")
                    xqbT = xqT_pool.tile([128, KC - KC8, 128], BF16,
                                         tag="xqbT", name=f"xqbT{rt}")
                    tp8 = psum_tp.tile([128, 512], F32, tag="tp",
                                       name=f"tp8_{rt}")
                    for j in range(4):
                        nc.tensor.matmul(
                            tp8[:, j * 128:(j + 1) * 128],
                            lhsT=xq8[:, j * 128:(j + 1) * 128],
                            rhs=ident[:])
                    nc.vector.tensor_copy(xq8T[:], tp8[:])
                    tpb = psum_tp.tile([128, 512], F32, tag="tp",
                                       name=f"tpb_{rt}")
                    for j in range(4):
                        nc.tensor.matmul(
                            tpb[:, j * 128:(j + 1) * 128],
                            lhsT=xqb[:, j * 128:(j + 1) * 128],
                            rhs=ident[:])
                    nc.scalar.copy(xqbT[:], tpb[:])
                xq8T_tiles.append(xq8T)
                xqbT_tiles.append(xqbT)
                xsc_tiles.append(xsc)

            # ---- matmul + rescale for one (row-tile, n-half) ----
            def emit_mm(rt, h):
                xq8T = xq8T_tiles[rt]
                xqbT = xqbT_tiles[rt]
                cs = cs_tiles[rt]
                with nc.named_scope("mm"):
                    pst = [
                        psum_mm.tile([128, 512], F32, tag="pmm",
                                     name=f"pmm_{rt}_{h}_{q}")
                        for q in range(NQ)
                    ]
                    ch = h * (N // NH)
                    # lhsT-outer so the stationary operand is loaded once
                    # per 4 matmuls; the 4 psum banks' accumulation chains
                    # advance in lockstep.
                    for jp in range(KC8 // 2):
                        for q in range(NQ):
                            c0 = ch + q * 512
                            nc.tensor.matmul(
                                pst[q][:],
                                lhsT=xq8T[:, 2 * jp:2 * jp + 2, :],
                                rhs=wT8[:, 2 * jp:2 * jp + 2, c0:c0 + 512],
                                start=(jp == 0), stop=False,
                                perf_mode=DR)
                    for j in range(KC - KC8):
                        for q in range(NQ):
                            c0 = ch + q * 512
                            nc.tensor.matmul(
                                pst[q][:],
                                lhsT=xqbT[:, j, :],
                                rhs=wTb[:, j, c0:c0 + 512],
                                start=False, stop=(j == KC - KC8 - 1))
                with nc.named_scope("out_scale"):
                    osbh = osb_pool.tile([128, N // NH], BF16, tag="osb",
                                         name=f"osb{rt}_{h}")
                    for q in range(NQ):
                        dst = osbh[:, q * 512:(q + 1) * 512]
                        if q == 0:
                            nc.vector.tensor_scalar(
                                out=dst, in0=pst[q][:], scalar1=cs[:, 0:1],
                                scalar2=None, op0=ALU.mult)
                        else:
                            nc.scalar.activation(
                                dst, pst[q][:], AF.Copy, scale=cs[:, 0:1])
                    eng = nc.sync if (rt + h) % 2 == 0 else nc.scalar
                    eng.dma_start(
                        out_d[rt * 128:(rt + 1) * 128,
                              h * 2048:(h + 1) * 2048],
                        osbh[:])

            # ---- emission schedule ----
            # w DMAs + AllReduce-independent w prep interleave with the first
            # x tiles; the AllReduce-gated DVE work lands as soon as possible;
            # mm chains join once their row tile exists.
            for kt in range(4):
                emit_w_pre(kt)
                emit_x(kt)
            wsb, thr2 = emit_scales()
            for kt in range(4, KC):
                emit_w_pre(kt)
                emit_w_post(kt - 4, thr2)
            for kt in range(4, KC):
                emit_w_post(kt, thr2)
            for rt in range(4, RT):
                emit_x(rt)
            with nc.named_scope("cs"):
                for rt in range(RT):
                    cs = cs_pool.tile([128, 1], F32, tag="cs", name=f"cs{rt}")
                    nc.vector.tensor_mul(cs[:], xsc_tiles[rt][:], wsb[:])
                    cs_tiles.append(cs)
            for rt in range(RT):
                emit_mm(rt, 0)
                emit_mm(rt, 1)

    nc.compile()
    return nc


def _ensure_ntff_hook():
    """Make trace=True work: bass_utils imports antenv.axon_hooks, which is
    not present in this image. Shim it and install the ctypes-based NTFF
    profiling hook against libaxon_pjrt.so (same recipe as trn_boot)."""
    import sys
    import types
    try:
        import antenv.axon_hooks  # noqa: F401
        return
    except ImportError:
        pass
    mod = types.ModuleType("antenv.axon_hooks")
    mod._hook = None
    mod.set_axon_ntff_profile_hook = lambda h: setattr(mod, "_hook", h)
    mod.get_axon_ntff_profile_hook = lambda: mod._hook
    sys.modules["antenv.axon_hooks"] = mod
    import antenv
    antenv.axon_hooks = mod
    try:
        from trn_agent_boot.trn_boot import _ntff_profile_via_ctypes
        hook = _ntff_profile_via_ctypes("/opt/axon/libaxon_pjrt.so")
        if hook is not None:
            mod._hook = hook
    except Exception as e:  # degrade to no-trace
        print(f"ntff hook install failed: {e}")
    # no S3 in this sandbox; keep artifacts local
    import concourse.bass_utils as bu
    bu.upload_artifacts = lambda tmpdir: f"local://{tmpdir}"


_NC_CACHE = {}


def kernel(x: np.ndarray, weight: np.ndarray, norm_weight: np.ndarray) -> np.ndarray:
    import os
    x = np.ascontiguousarray(x, dtype=np.float32)
    weight = np.ascontiguousarray(weight, dtype=np.float32)
    norm_weight = np.ascontiguousarray(norm_weight, dtype=np.float32)

    B, S, Kin = x.shape
    xf = x.reshape(-1, Kin)
    wT = np.ascontiguousarray(weight.T)       # (K, N), pure layout change
    g_is_ones = bool(np.all(norm_weight == 1.0))

    if g_is_ones not in _NC_CACHE:
        _NC_CACHE[g_is_ones] = build_nc(g_is_ones)
    nc = _NC_CACHE[g_is_ones]

    in_maps = []
    for i in range(N_CORES):
        m = {
            "x": xf[i * R:(i + 1) * R],
            "wt": wT,
            "wshard": wT[i * 128:(i + 1) * 128],
        }
        if not g_is_ones:
            m["g"] = norm_weight.reshape(1, Kin)
        in_maps.append(m)

    trace = bool(int(os.environ.get("BITLIN_TRACE", "0")))
    if trace:
        _ensure_ntff_hook()
    res = run_bass_kernel_spmd(
        nc, in_maps, core_ids=list(range(N_CORES)), trace=trace,
    )
    if trace:
        kernel.last_results = res
    out = np.concatenate([r["out"] for r in res.results], axis=0)
    return out.reshape(B, S, weight.shape[0]).astype(np.float32)


# revision 15
# speedup vs baseline: 1.2480x; 1.2480x over previous
"""BitLinear forward (RMSNorm -> int8 activation quant -> ternary weight quant
-> matmul -> rescale) on 8 Trainium2 NeuronCores.

Sharding: data-parallel over rows; each core gets 2048 rows and the full
weight. The weight arrives pre-transposed on the host (pure layout change):
wT = weight.T (1024, 4096), so k lands on SBUF partitions and no on-device
weight transposes are needed. w_scale = mean|w| comes from a per-core 128-row
shard of wT + a scalar AllReduce; every scalar derived from it (wsb, invb,
cs) is computed on the ACT engine so the AllReduce latency never head-of-line
blocks the DVE queue. Output is written bf16 (exact integers * f32 scale,
bf16-rounded; ~0.3% of the error budget), upcast to f32 on host.

Mixed-precision matmul: x_q are exact integers in [-128,127], w_t ternary
{-1,0,1}. k-chunks 0..3 are cast to fp8e4 (one RNE rounding of the integer
x_q; ternary weights are exact in fp8) and run as DoubleRow perf-mode
matmuls (2 k-rows per PE pass); chunks 4..7 stay bf16 (exact, mixed bf16 x
fp8 matmuls). Measured vs the fixed-seed reference: rel_err ~1.76e-2 < 2e-2.

The weight streams in as 64 [128,512] column strips, processed
column-window-major: ternarizing window w (cols 512w..512w+511, all 8
k-chunks) enables every row tile's matmul chain for that output window, so
the PE ramps ~10us after the AllReduce instead of waiting for all of w.
Ternarize is the proven magic-clip: u = min(w*invb, 1) (DVE), v =
max(u,-1)+C (DVE), wt = Copy(v, bias=-C) -> fp8 (ACT) == RNE(clip(ws,-1,1)).

x row stats run batched 4 row-tiles at a time ([128,4] chains).
Round-half-to-even is fp32 magic-constant (1.5*2^23) add/subtract. x
transposes are identity matmuls, 4 chunks per PSUM bank, one wide copy back.
"""

import numpy as np

import concourse.bass as bass
import concourse.mybir as mybir
import concourse.tile as tile
from concourse import bacc
from concourse.bass_utils import run_bass_kernel_spmd
from concourse.masks import make_identity
from concourse import bass_isa

F32 = mybir.dt.float32
BF16 = mybir.dt.bfloat16
FP8 = mybir.dt.float8e4
ALU = mybir.AluOpType
AF = mybir.ActivationFunctionType
DR = mybir.MatmulPerfMode.DoubleRow

N_CORES = 8
R_FULL, K, N = 16384, 1024, 4096
R = R_FULL // N_CORES          # 2048 rows per core
RT = R // 128                  # 16 row tiles per core
NB = 4                         # row tiles per stats batch
KC = K // 128                  # 8 k-chunks
KC8 = 4                        # chunks 0..3 are fp8 (DoubleRow), 4..7 bf16
NW = 8                         # 512-col output windows
NQ = 4                         # 512-wide psum tiles per half

C_MAGIC = 12582912.0           # 1.5 * 2^23: fp32 round-to-nearest-even trick
Q_EPS = 1e-5
NORM_EPS = 1e-6


def build_nc(g_is_ones: bool):
    nc = bacc.Bacc("TRN2", target_bir_lowering=False)

    x_d = nc.dram_tensor("x", [R, K], F32, kind="ExternalInput")
    wt_d = nc.dram_tensor("wt", [K, N], F32, kind="ExternalInput")
    wsh_d = nc.dram_tensor("wshard", [128, N], F32, kind="ExternalInput")
    cc_in = nc.dram_tensor("cc_in", [1, 1], F32)
    cc_out = nc.dram_tensor("cc_out", [1, 1], F32, addr_space="Shared")
    if not g_is_ones:
        g_d = nc.dram_tensor("g", [1, K], F32, kind="ExternalInput")
    out_d = nc.dram_tensor("out", [R, N], BF16, kind="ExternalOutput")

    with tile.TileContext(nc) as tc:
        with (
            tc.tile_pool(name="persist", bufs=1) as persist,
            tc.tile_pool(name="wkp", bufs=10) as wk_pool,
            tc.tile_pool(name="wup", bufs=3) as wu_pool,
            tc.tile_pool(name="xp", bufs=5) as x_pool,
            tc.tile_pool(name="xqTp", bufs=16) as xqT_pool,
            tc.tile_pool(name="csp", bufs=4) as cs_pool,
            tc.tile_pool(name="big", bufs=2) as big_pool,
            tc.tile_pool(name="bst", bufs=2) as bst_pool,
            tc.tile_pool(name="stats", bufs=2) as st_pool,
            tc.tile_pool(name="osbp", bufs=3) as osb_pool,
            tc.tile_pool(name="pmm", bufs=6, space="PSUM") as psum_mm,
            tc.tile_pool(name="ptp", bufs=2, space="PSUM") as psum_tp,
        ):
            # ---- constants ----
            ident = persist.tile([128, 128], BF16, tag="ident")
            make_identity(nc, ident[:])
            cb = persist.tile([128, 1], F32, tag="cb")
            nc.vector.memset(cb[:], C_MAGIC)
            epsb = persist.tile([128, 1], F32, tag="epsb")
            nc.vector.memset(epsb[:], Q_EPS)

            if not g_is_ones:
                g_row = persist.tile([1, K], F32, tag="g_row")
                nc.sync.dma_start(g_row[:], g_d[:])
                g_b = persist.tile([128, K], F32, tag="g_b")
                nc.gpsimd.partition_broadcast(g_b[:], g_row[0:1, :])

            # ternarized w^T, k on partitions, all fp8 (ternary is exact)
            wT8 = persist.tile([128, KC8, N], FP8, tag="wT8", name="wT8")
            wTb = persist.tile([128, KC - KC8, N], FP8, tag="wTb", name="wTb")

            # ---- w_scale sum from the 128-row wT shard; DMAs + collective
            # on the gpsimd queue so AllReduce latency blocks nothing ----
            with nc.named_scope("w_scale"):
                wsha = big_pool.tile([128, N], F32, tag="wsha", name="wsha")
                nc.gpsimd.dma_start(wsha[:], wsh_d[:])
                wabs = big_pool.tile([128, N], BF16, tag="wabs", name="wabs")
                wpart = st_pool.tile([128, 1], F32, tag="wpart")
                nc.scalar.activation(
                    wabs[:], wsha[:], AF.Abs, accum_out=wpart[:])
                wall = st_pool.tile([128, 1], F32, tag="wall")
                nc.gpsimd.partition_all_reduce(
                    wall[:], wpart[:], channels=128,
                    reduce_op=bass_isa.ReduceOp.add)
                nc.gpsimd.dma_start(cc_in[:], wall[0:1, :])
                nc.gpsimd.collective_compute(
                    "AllReduce", ALU.add,
                    replica_groups=[list(range(N_CORES))],
                    ins=[cc_in[:]], outs=[cc_out[:]],
                )
                wsum_g1 = st_pool.tile([1, 1], F32, tag="wsum_g1")
                nc.gpsimd.dma_start(wsum_g1[:], cc_out[:])
                wsum_g = persist.tile([128, 1], F32, tag="wsum_g")
                nc.gpsimd.partition_broadcast(wsum_g[:], wsum_g1[0:1, :])

            wsb = persist.tile([128, 1], F32, tag="wsb")
            speps = persist.tile([128, 1], F32, tag="speps")
            invb = persist.tile([128, 1], F32, tag="invb")

            def emit_scales():
                # ACT-only chain: wsb = mean|w|; invb = 1/(wsb+eps)
                nc.scalar.activation(
                    wsb[:], wsum_g[:], AF.Copy, scale=1.0 / (N * K))
                nc.scalar.activation(
                    speps[:], wsum_g[:], AF.Identity,
                    scale=1.0 / (N * K), bias=epsb[:, 0:1])
                nc.vector.reciprocal(invb[:], speps[:])

            # ---- w strips: DMA [128,512] col strips; ternarize per strip:
            # u = min(w*invb,1) (DVE), v = max(u,-1)+C (DVE),
            # wt = Copy(v, -C) -> fp8 (ACT); window-major emission ----
            def emit_w_strip(kt, w):
                c0 = w * 512
                with nc.named_scope("w_tern"):
                    wk = wk_pool.tile([128, 512], F32, tag="wk",
                                      name=f"wk{kt}_{w}")
                    nc.sync.dma_start(
                        wk[:], wt_d[kt * 128:(kt + 1) * 128, c0:c0 + 512])
                    u = wu_pool.tile([128, 512], F32, tag="wu",
                                     name=f"wu{kt}_{w}")
                    nc.vector.tensor_scalar(
                        out=u[:], in0=wk[:], scalar1=invb[:, 0:1],
                        scalar2=1.0, op0=ALU.mult, op1=ALU.min)
                    v = wu_pool.tile([128, 512], F32, tag="wv",
                                     name=f"wv{kt}_{w}")
                    nc.vector.tensor_scalar(
                        out=v[:], in0=u[:], scalar1=-1.0,
                        scalar2=C_MAGIC, op0=ALU.max, op1=ALU.add)
                    if kt < KC8:
                        dst = wT8[:, kt, c0:c0 + 512]
                    else:
                        dst = wTb[:, kt - KC8, c0:c0 + 512]
                    nc.scalar.activation(dst, v[:], AF.Copy, bias=-C_MAGIC)

            # ---- x quantization, batched NB row tiles at a time ----
            xq8T_tiles = []
            xqbT_tiles = []
            cs_tiles = []   # cs_tiles[b][:, i] is cs for rt = b*NB + i

            def emit_batch(b):
                rts = range(b * NB, (b + 1) * NB)
                with nc.named_scope("x_quant"):
                    xgs = []
                    ssqb = bst_pool.tile([128, NB], F32, tag="ssqb",
                                         name=f"ssqb{b}")
                    amb = bst_pool.tile([128, NB], F32, tag="amb",
                                        name=f"amb{b}")
                    for i, rt in enumerate(rts):
                        xt = x_pool.tile([128, K], F32, tag="xt",
                                         name=f"xt{rt}")
                        nc.scalar.dma_start(
                            xt[:], x_d[rt * 128:(rt + 1) * 128, :])
                        if not g_is_ones:
                            xg = big_pool.tile([128, K], F32, tag="xg",
                                               name=f"xg{rt}")
                            nc.vector.tensor_mul(xg[:], xt[:], g_b[:])
                        else:
                            xg = xt
                        xgs.append(xg)
                        xsq = big_pool.tile([128, K], FP8, tag="xsq",
                                            name=f"xsq{rt}")
                        nc.vector.scalar_tensor_tensor(
                            out=xsq[:], in0=xt[:], scalar=1.0, in1=xt[:],
                            op0=ALU.mult, op1=ALU.mult,
                            accum_out=ssqb[:, i:i + 1])
                        nc.vector.tensor_reduce(
                            amb[:, i:i + 1], xg[:], axis=mybir.AxisListType.X,
                            op=ALU.max, apply_absolute_value=True)

                    # batched scalar chain on [128, NB]:
                    # rs = 1/sqrt(ms+eps) with one Newton step on sqrt
                    ms = bst_pool.tile([128, NB], F32, tag="ms")
                    nc.vector.tensor_scalar(
                        out=ms[:], in0=ssqb[:], scalar1=1.0 / K,
                        scalar2=NORM_EPS, op0=ALU.mult, op1=ALU.add)
                    s0 = bst_pool.tile([128, NB], F32, tag="s0")
                    nc.scalar.sqrt(s0[:], ms[:])
                    r0 = bst_pool.tile([128, NB], F32, tag="r0")
                    nc.vector.reciprocal(r0[:], s0[:])
                    t0 = bst_pool.tile([128, NB], F32, tag="t0")
                    nc.vector.tensor_mul(t0[:], ms[:], r0[:])
                    t1 = bst_pool.tile([128, NB], F32, tag="t1")
                    nc.vector.tensor_add(t1[:], t0[:], s0[:])
                    s1 = bst_pool.tile([128, NB], F32, tag="s1")
                    nc.vector.tensor_scalar(
                        out=s1[:], in0=t1[:], scalar1=0.5,
                        scalar2=None, op0=ALU.mult)
                    rs = bst_pool.tile([128, NB], F32, tag="rs")
                    nc.vector.reciprocal(rs[:], s1[:])
                    axr = bst_pool.tile([128, NB], F32, tag="axr")
                    nc.vector.tensor_mul(axr[:], amb[:], rs[:])
                    xscb = bst_pool.tile([128, NB], F32, tag="xscb")
                    nc.vector.tensor_scalar(
                        out=xscb[:], in0=axr[:], scalar1=1.0 / 127.0,
                        scalar2=None, op0=ALU.mult)
                    sx = bst_pool.tile([128, NB], F32, tag="sx")
                    nc.vector.tensor_scalar(
                        out=sx[:], in0=axr[:], scalar1=1.0 / 127.0,
                        scalar2=Q_EPS, op0=ALU.mult, op1=ALU.add)
                    dx = bst_pool.tile([128, NB], F32, tag="dx")
                    nc.vector.reciprocal(dx[:], sx[:])
                    srowb = bst_pool.tile([128, NB], F32, tag="srowb")
                    nc.vector.tensor_mul(srowb[:], rs[:], dx[:])
                    # cs = xsc * wsb on ACT (Copy with per-partition scale)
                    csb = cs_pool.tile([128, NB], F32, tag="csb",
                                       name=f"csb{b}")
                    nc.scalar.activation(
                        csb[:], xscb[:], AF.Copy, scale=wsb[:, 0:1])
                    cs_tiles.append(csb)

                    for i, rt in enumerate(rts):
                        xg = xgs[i]
                        # x_q = RNE(xg*srow) via +C (ACT) then -C with the
                        # dtype conversion: chunks 0..3 fp8 (ACT), 4..7
                        # bf16 (DVE)
                        ux = big_pool.tile([128, K], F32, tag="ux",
                                           name=f"ux{rt}")
                        nc.scalar.activation(
                            ux[:], xg[:], AF.Identity,
                            bias=cb[:, 0:1], scale=srowb[:, i:i + 1])
                        xq8 = big_pool.tile([128, K // 2], FP8, tag="xq8",
                                            name=f"xq8{rt}")
                        nc.scalar.activation(
                            xq8[:], ux[:, :K // 2], AF.Copy, bias=-C_MAGIC)
                        xqb = big_pool.tile([128, K // 2], BF16, tag="xqb",
                                            name=f"xqb{rt}")
                        nc.vector.tensor_scalar(
                            out=xqb[:], in0=ux[:, K // 2:], scalar1=C_MAGIC,
                            scalar2=None, op0=ALU.subtract)

                        xq8T = xqT_pool.tile([128, KC8, 128], FP8,
                                             tag="xq8T", name=f"xq8T{rt}")
                        xqbT = xqT_pool.tile([128, KC - KC8, 128], BF16,
                                             tag="xqbT", name=f"xqbT{rt}")
                        tp8 = psum_tp.tile([128, 512], F32, tag="tp",
                                           name=f"tp8_{rt}")
                        for j in range(4):
                            nc.tensor.matmul(
                                tp8[:, j * 128:(j + 1) * 128],
                                lhsT=xq8[:, j * 128:(j + 1) * 128],
                                rhs=ident[:])
                        nc.vector.tensor_copy(xq8T[:], tp8[:])
                        tpb = psum_tp.tile([128, 512], F32, tag="tp",
                                           name=f"tpb_{rt}")
                        for j in range(4):
                            nc.tensor.matmul(
                                tpb[:, j * 128:(j + 1) * 128],
                                lhsT=xqb[:, j * 128:(j + 1) * 128],
                                rhs=ident[:])
                        nc.scalar.copy(xqbT[:], tpb[:])
                        xq8T_tiles.append(xq8T)
                        xqbT_tiles.append(xqbT)

            # ---- matmul + rescale for one (row-tile, 1024-col group) ----
            # 2 windows per group: each stationary operand feeds 2
            # back-to-back matmuls so the next LDWEIGHTS hides under them.
            def emit_mm(rt, g):
                xq8T = xq8T_tiles[rt]
                xqbT = xqbT_tiles[rt]
                cs = cs_tiles[rt // NB][:, rt % NB:rt % NB + 1]
                with nc.named_scope("mm"):
                    pst = [
                        psum_mm.tile([128, 512], F32, tag="pmm",
                                     name=f"pmm_{rt}_{g}_{q}")
                        for q in range(2)
                    ]
                    for jp in range(KC8 // 2):
                        for q in range(2):
                            c0 = (2 * g + q) * 512
                            nc.tensor.matmul(
                                pst[q][:],
                                lhsT=xq8T[:, 2 * jp:2 * jp + 2, :],
                                rhs=wT8[:, 2 * jp:2 * jp + 2, c0:c0 + 512],
                                start=(jp == 0), stop=False,
                                perf_mode=DR)
                    for j in range(KC - KC8):
                        for q in range(2):
                            c0 = (2 * g + q) * 512
                            nc.tensor.matmul(
                                pst[q][:],
                                lhsT=xqbT[:, j, :],
                                rhs=wTb[:, j, c0:c0 + 512],
                                start=False, stop=(j == KC - KC8 - 1))
                with nc.named_scope("out_scale"):
                    osbh = osb_pool.tile([128, 1024], BF16, tag="osb",
                                         name=f"osb{rt}_{g}")
                    nc.vector.tensor_scalar(
                        out=osbh[:, 0:512], in0=pst[0][:], scalar1=cs,
                        scalar2=None, op0=ALU.mult)
                    nc.scalar.activation(
                        osbh[:, 512:1024], pst[1][:], AF.Copy, scale=cs)
                    eng = nc.sync if (rt + g) % 2 == 0 else nc.scalar
                    eng.dma_start(
                        out_d[rt * 128:(rt + 1) * 128,
                              2 * g * 512:(2 * g + 2) * 512],
                        osbh[:])

            # ---- emission schedule ----
            emit_batch(0)
            emit_batch(1)
            emit_scales()
            # w strips window-major; first windows unblock mm chains early
            for w in range(2):
                for kt in range(KC):
                    emit_w_strip(kt, w)
            emit_batch(2)
            for w in range(2, 5):
                for kt in range(KC):
                    emit_w_strip(kt, w)
            emit_batch(3)
            for w in range(5, NW):
                for kt in range(KC):
                    emit_w_strip(kt, w)
            # mm chains: group-major so early column groups drain first,
            # row tiles cycling within each group
            for g in range(NW // 2):
                for rt in range(RT):
                    emit_mm(rt, g)

    nc.compile()
    return nc


def _ensure_ntff_hook():
    """Make trace=True work: bass_utils imports antenv.axon_hooks, which is
    not present in this image. Shim it and install the ctypes-based NTFF
    profiling hook against libaxon_pjrt.so (same recipe as trn_boot)."""
    import sys
    import types
    try:
        import antenv.axon_hooks  # noqa: F401
        return
    except ImportError:
        pass
    mod = types.ModuleType("antenv.axon_hooks")
    mod._hook = None
    mod.set_axon_ntff_profile_hook = lambda h: setattr(mod, "_hook", h)
    mod.get_axon_ntff_profile_hook = lambda: mod._hook
    sys.modules["antenv.axon_hooks"] = mod
    import antenv
    antenv.axon_hooks = mod
    try:
        from trn_agent_boot.trn_boot import _ntff_profile_via_ctypes
        hook = _ntff_profile_via_ctypes("/opt/axon/libaxon_pjrt.so")
        if hook is not None:
            mod._hook = hook
    except Exception as e:  # degrade to no-trace
        print(f"ntff hook install failed: {e}")
    # no S3 in this sandbox; keep artifacts local
    import concourse.bass_utils as bu
    bu.upload_artifacts = lambda tmpdir: f"local://{tmpdir}"


_NC_CACHE = {}


def kernel(x: np.ndarray, weight: np.ndarray, norm_weight: np.ndarray) -> np.ndarray:
    import os
    x = np.ascontiguousarray(x, dtype=np.float32)
    weight = np.ascontiguousarray(weight, dtype=np.float32)
    norm_weight = np.ascontiguousarray(norm_weight, dtype=np.float32)

    B, S, Kin = x.shape
    xf = x.reshape(-1, Kin)
    wT = np.ascontiguousarray(weight.T)       # (K, N), pure layout change
    g_is_ones = bool(np.all(norm_weight == 1.0))

    if g_is_ones not in _NC_CACHE:
        _NC_CACHE[g_is_ones] = build_nc(g_is_ones)
    nc = _NC_CACHE[g_is_ones]

    in_maps = []
    for i in range(N_CORES):
        m = {
            "x": xf[i * R:(i + 1) * R],
            "wt": wT,
            "wshard": wT[i * 128:(i + 1) * 128],
        }
        if not g_is_ones:
            m["g"] = norm_weight.reshape(1, Kin)
        in_maps.append(m)

    trace = bool(int(os.environ.get("BITLIN_TRACE", "0")))
    if trace:
        _ensure_ntff_hook()
    res = run_bass_kernel_spmd(
        nc, in_maps, core_ids=list(range(N_CORES)), trace=trace,
    )
    if trace:
        kernel.last_results = res
    out = np.concatenate([r["out"] for r in res.results], axis=0)
    return out.reshape(B, S, weight.shape[0]).astype(np.float32)
